# revision 5
# baseline (speedup 1.0000x reference)
"""TRN2 Bass kernel for nn_AVNNType1Linear.

Reference computation (B=2048, D_in=D_out=4096):
    act, carry = x[..., 0], x[..., 1]                  # x: [B, D_in, 2] f32
    act_out    = relu(act @ W.T + b)                   # [B, D_out]
    val        = 0.5*(mean(act, -1) + mean(carry, -1)) # [B]
    out        = stack([act_out, broadcast(val)], -1)  # [B, D_out, 2]

Distribution over 8 NeuronCores: 4-way data-parallel over batch x 2-way
tensor-parallel over output features (minimizes per-core HBM traffic:
xaT 8.4MB + xc 8.4MB + W.T-half 33.6MB + out 8.4MB ~= 59MB/core).

Per-core kernel: single-pass float32r matmul (full PE rate, ~2**-13
relative rounding vs ~2**-8 for bf16). The activator comes in host-
transposed ([D_in, B_loc]) so contraction sits on the partition dim with
clean DMAs; bias is folded into PSUM via a K=1 ones-row matmul; the
activator row-sums fall out of N=1 ones-column matmuls accumulated in
PSUM alongside the main GEMM; carry row-sums are a DVE free-dim reduce
of the naturally-laid-out carry channel. The [b, (o,ch)] interleaved
output tile is assembled in SBUF so the store DMA writes contiguous 4KB
rows.
"""

import os

import numpy as np

import concourse.mybir as mybir
import concourse.tile as tile
from concourse import bacc
from concourse.bass_utils import run_bass_kernel_spmd

B, D = 2048, 4096          # batch, D_in == D_out
M_SHARDS, F_SHARDS = 4, 2  # batch x feature grid over 8 cores
B_LOC = B // M_SHARDS      # 512 batch rows per core
O_LOC = D // F_SHARDS      # 2048 output features per core
KT = D // 128              # 32 contraction tiles
OT = O_LOC // 512          # 4 output tiles of 512
JT = B_LOC // 128          # 4 batch tiles of 128
KG = 4                     # activator SBUF tiles (groups of KT//KG k-tiles)
KPG = KT // KG


def _build():
    dt = mybir.dt
    nc = bacc.Bacc("TRN2", target_bir_lowering=False, debug=False)
    xaT = nc.dram_tensor("xaT", [D, B_LOC], dt.float32r, kind="ExternalInput").ap()
    xc = nc.dram_tensor("xc", [B_LOC, D], dt.float32, kind="ExternalInput").ap()
    wT = nc.dram_tensor("wT", [D, O_LOC], dt.float32r, kind="ExternalInput").ap()
    bias = nc.dram_tensor("bias", [1, O_LOC], dt.float32r, kind="ExternalInput").ap()
    ones = nc.dram_tensor("ones", [128, 128], dt.float32r, kind="ExternalInput").ap()
    out = nc.dram_tensor(
        "out", [B_LOC, O_LOC, 2], dt.float32, kind="ExternalOutput"
    ).ap()

    with tile.TileContext(nc) as tc:
        with (
            tc.tile_pool(name="persist", bufs=1) as persist,
            tc.tile_pool(name="wstream", bufs=6) as wpool,
            tc.tile_pool(name="xcpool", bufs=2) as xcpool,
            tc.tile_pool(name="opool", bufs=4) as opool,
            tc.tile_pool(name="small", bufs=1) as small,
            tc.tile_pool(name="ps", bufs=6, space="PSUM") as pspool,
            tc.tile_pool(name="psval", bufs=1, space="PSUM") as psvalpool,
        ):
            # --- persistent loads -----------------------------------
            ones_sb = persist.tile([128, 128], dt.float32r)
            nc.sync.dma_start(out=ones_sb, in_=ones)
            bias_sb = persist.tile([1, O_LOC], dt.float32r)
            nc.sync.dma_start(out=bias_sb, in_=bias)

            # whole activator shard, [i%128, kt, b] layout, in KG chunks
            act_g = []
            for g in range(KG):
                t = persist.tile([128, KPG, B_LOC], dt.float32r, tag=f"act{g}")
                nc.sync.dma_start(
                    out=t,
                    in_=xaT[g * KPG * 128 : (g + 1) * KPG * 128, :].rearrange(
                        "(kt p) b -> p kt b", p=128
                    ),
                )
                act_g.append(t)

            def act_tile(k, j):
                return act_g[k // KPG][:, k % KPG, j * 128 : (j + 1) * 128]

            # --- carry row sums (DVE) -------------------------------
            csum_sb = small.tile([128, JT], dt.float32)
            with nc.named_scope("carry_sums"):
                for j in range(JT):
                    xc_t = xcpool.tile([128, D], dt.float32, tag="xc")
                    nc.sync.dma_start(out=xc_t, in_=xc[j * 128 : (j + 1) * 128, :])
                    nc.vector.reduce_sum(
                        csum_sb[:, j : j + 1], xc_t, axis=mybir.AxisListType.X
                    )

            # --- main GEMM + act row sums ---------------------------
            # fp32r matmuls need an even moving/dst free count, so the
            # row-sum matmuls use N=2 (two ones columns; col 2j+1 is a
            # duplicate of 2j and ignored).
            psum_val = psvalpool.tile([128, 2 * JT], dt.float32)
            val_sb = small.tile([128, JT], dt.float32)
            for o in range(OT):
                o_sl = slice(o * 512, (o + 1) * 512)
                ps = [
                    pspool.tile([128, 512], dt.float32, tag="ps", name=f"ps_{o}_{j}")
                    for j in range(JT)
                ]
                for k in range(KT):
                    w_t = wpool.tile([128, 512], dt.float32r, tag="wt")
                    nc.sync.dma_start(
                        out=w_t, in_=wT[k * 128 : (k + 1) * 128, o_sl]
                    )
                    for j in range(JT):
                        nc.tensor.matmul(
                            ps[j], act_tile(k, j), w_t,
                            start=(k == 0), stop=(k == KT - 1),
                        )
                        if k == 0:
                            # bias: ones-row (K=1) x bias-row accumulate
                            nc.tensor.matmul(
                                ps[j], ones_sb[0:1, :], bias_sb[0:1, o_sl],
                                start=False, stop=False,
                            )
                        if o == 0:
                            # activator row sums: act_tile.T @ ones_cols
                            nc.tensor.matmul(
                                psum_val[:, 2 * j : 2 * j + 2],
                                act_tile(k, j), ones_sb[:, 0:2],
                                start=(k == 0), stop=(k == KT - 1),
                            )
                if o == 0:
                    # val = (act_sum + carry_sum) / (2*D)
                    for j in range(JT):
                        nc.vector.tensor_scalar(
                            val_sb[:, j : j + 1], psum_val[:, 2 * j : 2 * j + 1],
                            csum_sb[:, j : j + 1], 1.0 / (2 * D),
                            op0=mybir.AluOpType.add, op1=mybir.AluOpType.mult,
                        )
                # --- epilogue: relu + carry broadcast + store -------
                for j in range(JT):
                    out_t = opool.tile([128, 512, 2], dt.float32, tag="out")
                    nc.vector.tensor_scalar_max(out_t[:, :, 0], ps[j], 0.0)
                    nc.vector.tensor_scalar(
                        out_t[:, :, 1], ps[j], 0.0, val_sb[:, j : j + 1],
                        op0=mybir.AluOpType.mult, op1=mybir.AluOpType.add,
                    )
                    nc.sync.dma_start(
                        out=out[j * 128 : (j + 1) * 128, o_sl, :], in_=out_t
                    )
    nc.compile()
    return nc


_ONES = None


def _shard_inputs(x, W, b):
    global _ONES
    if _ONES is None:
        _ONES = np.ones((128, 128), dtype=np.float32)
    x = np.ascontiguousarray(x, dtype=np.float32)
    W = np.asarray(W, dtype=np.float32)
    b = np.asarray(b, dtype=np.float32)
    wT_shards = [
        np.ascontiguousarray(W[c * O_LOC : (c + 1) * O_LOC, :].T)
        for c in range(F_SHARDS)
    ]
    bias_shards = [
        np.ascontiguousarray(b[c * O_LOC : (c + 1) * O_LOC]).reshape(1, O_LOC)
        for c in range(F_SHARDS)
    ]
    in_maps = []
    for core in range(M_SHARDS * F_SHARDS):
        r, c = core % M_SHARDS, core // M_SHARDS
        b_sl = slice(r * B_LOC, (r + 1) * B_LOC)
        in_maps.append(
            dict(
                xaT=np.ascontiguousarray(x[b_sl, :, 0].T),
                xc=np.ascontiguousarray(x[b_sl, :, 1]),
                wT=wT_shards[c],
                bias=bias_shards[c],
                ones=_ONES,
            )
        )
    return in_maps


def _gather(results):
    out = np.empty((B, D, 2), dtype=np.float32)
    for core, r in enumerate(results):
        m, c = core % M_SHARDS, core // M_SHARDS
        out[m * B_LOC : (m + 1) * B_LOC, c * O_LOC : (c + 1) * O_LOC, :] = r["out"]
    return out


def _run(x, W, b, trace=False, **spmd_kwargs):
    in_maps = _shard_inputs(x, W, b)
    nc = _build()
    res = run_bass_kernel_spmd(
        nc, in_maps, core_ids=list(range(8)), trace=trace, **spmd_kwargs
    )
    return _gather(res.results), res


def kernel(x, W, b):
    out, _ = _run(x, W, b, trace=False)
    return out


# revision 10
# speedup vs baseline: 1.0505x; 1.0505x over previous
"""TRN2 Bass kernel for nn_AVNNType1Linear.

Reference computation (B=2048, D_in=D_out=4096):
    act, carry = x[..., 0], x[..., 1]                  # x: [B, D_in, 2] f32
    act_out    = relu(act @ W.T + b)                   # [B, D_out]
    val        = 0.5*(mean(act, -1) + mean(carry, -1)) # [B]
    out        = stack([act_out, broadcast(val)], -1)  # [B, D_out, 2]

Distribution over 8 NeuronCores: 4-way data-parallel over batch x 2-way
tensor-parallel over output features (minimizes per-core HBM traffic:
xaT 8.4MB + xc 8.4MB + W.T-half 33.6MB + out 8.4MB ~= 59MB/core).

Per-core kernel: single-pass float32r matmul (full PE rate, ~2**-13
relative rounding vs ~2**-8 for bf16). The activator comes in host-
transposed ([D_in, B_loc]) so contraction sits on the partition dim with
clean DMAs; bias is folded into PSUM via a K=1 ones-row matmul; the
activator row-sums fall out of N=1 ones-column matmuls accumulated in
PSUM alongside the main GEMM; carry row-sums are a DVE free-dim reduce
of the naturally-laid-out carry channel. The [b, (o,ch)] interleaved
output tile is assembled in SBUF so the store DMA writes contiguous 4KB
rows.
"""

import os

import numpy as np

import concourse.mybir as mybir
import concourse.tile as tile
from concourse import bacc
from concourse.bass_utils import run_bass_kernel_spmd

B, D = 2048, 4096          # batch, D_in == D_out
M_SHARDS, F_SHARDS = 4, 2  # batch x feature grid over 8 cores
B_LOC = B // M_SHARDS      # 512 batch rows per core
O_LOC = D // F_SHARDS      # 2048 output features per core
KT = D // 128              # 32 contraction tiles
OT = O_LOC // 512          # 4 output tiles of 512
JT = B_LOC // 128          # 4 batch tiles of 128
KG = 4                     # activator SBUF tiles (groups of KT//KG k-tiles)
KPG = KT // KG


def _build():
    dt = mybir.dt
    nc = bacc.Bacc("TRN2", target_bir_lowering=False, debug=False)
    xaT = nc.dram_tensor("xaT", [D, B_LOC], dt.float32r, kind="ExternalInput").ap()
    xc = nc.dram_tensor("xc", [B_LOC, D], dt.float32, kind="ExternalInput").ap()
    wT = nc.dram_tensor("wT", [D, O_LOC], dt.float32r, kind="ExternalInput").ap()
    bias = nc.dram_tensor("bias", [1, O_LOC], dt.float32r, kind="ExternalInput").ap()
    ones = nc.dram_tensor("ones", [128, 128], dt.float32r, kind="ExternalInput").ap()
    out = nc.dram_tensor(
        "out", [B_LOC, O_LOC, 2], dt.float32, kind="ExternalOutput"
    ).ap()
    debug = os.environ.get("KERNEL_DEBUG") == "1"
    if debug:
        dbg = nc.dram_tensor(
            "dbg", [128, 4 * JT + 2 * JT], dt.float32, kind="ExternalOutput"
        ).ap()

    with tile.TileContext(nc) as tc:
        with (
            tc.tile_pool(name="persist", bufs=1) as persist,
            tc.tile_pool(name="wstream", bufs=6) as wpool,
            tc.tile_pool(name="xcpool", bufs=4) as xcpool,
            tc.tile_pool(name="opool", bufs=4) as opool,
            tc.tile_pool(name="small", bufs=1) as small,
            tc.tile_pool(name="ps", bufs=6, space="PSUM") as pspool,
            tc.tile_pool(name="psval", bufs=1, space="PSUM") as psvalpool,
        ):
            # --- persistent loads -----------------------------------
            ones_sb = persist.tile([128, 128], dt.float32r)
            nc.sync.dma_start(out=ones_sb, in_=ones)
            bias_sb = persist.tile([1, O_LOC], dt.float32r)
            nc.sync.dma_start(out=bias_sb, in_=bias)

            # whole activator shard, [i%128, kt, b] layout, in KG chunks
            act_g = []
            for g in range(KG):
                t = persist.tile([128, KPG, B_LOC], dt.float32r, tag=f"act{g}")
                nc.sync.dma_start(
                    out=t,
                    in_=xaT[g * KPG * 128 : (g + 1) * KPG * 128, :].rearrange(
                        "(kt p) b -> p kt b", p=128
                    ),
                )
                act_g.append(t)

            def act_tile(k, j):
                return act_g[k // KPG][:, k % KPG, j * 128 : (j + 1) * 128]

            # --- carry row sums (DVE) -------------------------------
            csum_sb = small.tile([128, JT], dt.float32)
            with nc.named_scope("carry_sums"):
                for j in range(JT):
                    xc_t = xcpool.tile([128, D], dt.float32, tag="xc")
                    nc.sync.dma_start(out=xc_t, in_=xc[j * 128 : (j + 1) * 128, :])
                    nc.vector.reduce_sum(
                        csum_sb[:, j : j + 1], xc_t, axis=mybir.AxisListType.X
                    )

            # --- main GEMM + act row sums ---------------------------
            # fp32r matmuls need an even moving/dst free count, so the
            # row-sum matmuls use N=2 (two ones columns; col 2j+1 is a
            # duplicate of 2j and ignored).
            psum_val = psvalpool.tile([128, 4 * JT], dt.float32)
            val_sb = small.tile([128, JT], dt.float32)
            for o in range(OT):
                o_sl = slice(o * 512, (o + 1) * 512)
                ps = [
                    pspool.tile([128, 512], dt.float32, tag="ps", name=f"ps_{o}_{j}")
                    for j in range(JT)
                ]
                for k in range(KT):
                    w_t = wpool.tile([128, 512], dt.float32r, tag="wt")
                    nc.sync.dma_start(
                        out=w_t, in_=wT[k * 128 : (k + 1) * 128, o_sl]
                    )
                    for j in range(JT):
                        nc.tensor.matmul(
                            ps[j], act_tile(k, j), w_t,
                            start=(k == 0), stop=(k == KT - 1),
                        )
                        if k == 0:
                            # bias: ones-row (K=1) x bias-row accumulate
                            nc.tensor.matmul(
                                ps[j], ones_sb[0:1, :], bias_sb[0:1, o_sl],
                                start=False, stop=False,
                            )
                        if o == 0:
                            # activator row sums: act_tile.T @ ones_cols.
                            # start=True clears has_written for the WHOLE
                            # bank, so only the first sum-MM may set it —
                            # later js overwrite-on-unset and accumulate
                            # from there.
                            nc.tensor.matmul(
                                psum_val[:, 4 * j : 4 * j + 2],
                                act_tile(k, j), ones_sb[:, 0:2],
                                start=(k == 0 and j == 0),
                                stop=(k == KT - 1 and j == JT - 1),
                                skip_group_check=True,
                            )
                if o == 0:
                    # val = (act_sum + carry_sum) / (2*D)
                    for j in range(JT):
                        nc.vector.tensor_scalar(
                            val_sb[:, j : j + 1], psum_val[:, 4 * j : 4 * j + 1],
                            csum_sb[:, j : j + 1], 1.0 / (2 * D),
                            op0=mybir.AluOpType.add, op1=mybir.AluOpType.mult,
                        )
                    if debug:
                        dbg_sb = small.tile([128, 6 * JT], dt.float32)
                        nc.vector.tensor_copy(dbg_sb[:, : 4 * JT], psum_val)
                        nc.vector.tensor_copy(
                            dbg_sb[:, 4 * JT : 5 * JT], csum_sb
                        )
                        nc.vector.tensor_copy(dbg_sb[:, 5 * JT :], val_sb)
                        nc.sync.dma_start(out=dbg, in_=dbg_sb)
                # --- epilogue: relu + carry broadcast + store -------
                for j in range(JT):
                    out_t = opool.tile([128, 512, 2], dt.float32, tag="out")
                    nc.vector.tensor_scalar_max(out_t[:, :, 0], ps[j], 0.0)
                    nc.vector.tensor_scalar(
                        out_t[:, :, 1], ps[j], 0.0, val_sb[:, j : j + 1],
                        op0=mybir.AluOpType.mult, op1=mybir.AluOpType.add,
                    )
                    nc.sync.dma_start(
                        out=out[j * 128 : (j + 1) * 128, o_sl, :], in_=out_t
                    )
    nc.compile()
    return nc


_ONES = None


def _shard_inputs(x, W, b):
    global _ONES
    if _ONES is None:
        _ONES = np.ones((128, 128), dtype=np.float32)
    x = np.ascontiguousarray(x, dtype=np.float32)
    W = np.asarray(W, dtype=np.float32)
    b = np.asarray(b, dtype=np.float32)
    wT_shards = [
        np.ascontiguousarray(W[c * O_LOC : (c + 1) * O_LOC, :].T)
        for c in range(F_SHARDS)
    ]
    bias_shards = [
        np.ascontiguousarray(b[c * O_LOC : (c + 1) * O_LOC]).reshape(1, O_LOC)
        for c in range(F_SHARDS)
    ]
    in_maps = []
    for core in range(M_SHARDS * F_SHARDS):
        r, c = core % M_SHARDS, core // M_SHARDS
        b_sl = slice(r * B_LOC, (r + 1) * B_LOC)
        in_maps.append(
            dict(
                xaT=np.ascontiguousarray(x[b_sl, :, 0].T),
                xc=np.ascontiguousarray(x[b_sl, :, 1]),
                wT=wT_shards[c],
                bias=bias_shards[c],
                ones=_ONES,
            )
        )
    return in_maps


def _gather(results):
    out = np.empty((B, D, 2), dtype=np.float32)
    for core, r in enumerate(results):
        m, c = core % M_SHARDS, core // M_SHARDS
        out[m * B_LOC : (m + 1) * B_LOC, c * O_LOC : (c + 1) * O_LOC, :] = r["out"]
    return out


def _run(x, W, b, trace=False, **spmd_kwargs):
    in_maps = _shard_inputs(x, W, b)
    nc = _build()
    res = run_bass_kernel_spmd(
        nc, in_maps, core_ids=list(range(8)), trace=trace, **spmd_kwargs
    )
    return _gather(res.results), res


def kernel(x, W, b):
    out, _ = _run(x, W, b, trace=False)
    return out


# revision 12
# speedup vs baseline: 1.3797x; 1.3134x over previous
"""TRN2 Bass kernel for nn_AVNNType1Linear.

Reference computation (B=2048, D_in=D_out=4096):
    act, carry = x[..., 0], x[..., 1]                  # x: [B, D_in, 2] f32
    act_out    = relu(act @ W.T + b)                   # [B, D_out]
    val        = 0.5*(mean(act, -1) + mean(carry, -1)) # [B]
    out        = stack([act_out, broadcast(val)], -1)  # [B, D_out, 2]

Distribution over 8 NeuronCores: 4-way data-parallel over batch x 2-way
tensor-parallel over output features (minimizes per-core HBM traffic:
xaT 8.4MB + xc 8.4MB + W.T-half 33.6MB + out 8.4MB ~= 59MB/core).

Per-core kernel: single-pass float32r matmul (full PE rate, ~2**-13
relative rounding vs ~2**-8 for bf16). The activator comes in host-
transposed ([D_in, B_loc]) so contraction sits on the partition dim with
clean DMAs; bias is folded into PSUM via a K=1 ones-row matmul; the
activator row-sums fall out of N=1 ones-column matmuls accumulated in
PSUM alongside the main GEMM; carry row-sums are a DVE free-dim reduce
of the naturally-laid-out carry channel. The [b, (o,ch)] interleaved
output tile is assembled in SBUF so the store DMA writes contiguous 4KB
rows.
"""

import os

import numpy as np

import concourse.mybir as mybir
import concourse.tile as tile
from concourse import bacc
from concourse.bass_utils import run_bass_kernel_spmd

B, D = 2048, 4096          # batch, D_in == D_out
M_SHARDS, F_SHARDS = 4, 2  # batch x feature grid over 8 cores
B_LOC = B // M_SHARDS      # 512 batch rows per core
O_LOC = D // F_SHARDS      # 2048 output features per core
KT = D // 128              # 32 contraction tiles
OT = O_LOC // 512          # 4 output tiles of 512
JT = B_LOC // 128          # 4 batch tiles of 128
KG = 4                     # activator SBUF tiles (groups of KT//KG k-tiles)
KPG = KT // KG


MM_DTYPE = os.environ.get("MM_DTYPE", "float16")


def _build():
    dt = mybir.dt
    mmdt = getattr(dt, MM_DTYPE)
    nc = bacc.Bacc("TRN2", target_bir_lowering=False, debug=False)
    xaT = nc.dram_tensor("xaT", [D, B_LOC], mmdt, kind="ExternalInput").ap()
    xc = nc.dram_tensor("xc", [B_LOC, D], dt.float32, kind="ExternalInput").ap()
    wT = nc.dram_tensor("wT", [D, O_LOC], mmdt, kind="ExternalInput").ap()
    bias = nc.dram_tensor("bias", [1, O_LOC], mmdt, kind="ExternalInput").ap()
    ones = nc.dram_tensor("ones", [128, 128], mmdt, kind="ExternalInput").ap()
    out = nc.dram_tensor(
        "out", [B_LOC, O_LOC, 2], dt.float32, kind="ExternalOutput"
    ).ap()
    debug = os.environ.get("KERNEL_DEBUG") == "1"
    if debug:
        dbg = nc.dram_tensor(
            "dbg", [128, 4 * JT + 2 * JT], dt.float32, kind="ExternalOutput"
        ).ap()

    with tile.TileContext(nc) as tc:
        with (
            tc.tile_pool(name="persist", bufs=1) as persist,
            tc.tile_pool(name="wstream", bufs=6) as wpool,
            tc.tile_pool(name="xcpool", bufs=4) as xcpool,
            tc.tile_pool(name="opool", bufs=4) as opool,
            tc.tile_pool(name="small", bufs=1) as small,
            tc.tile_pool(name="ps", bufs=6, space="PSUM") as pspool,
            tc.tile_pool(name="psval", bufs=1, space="PSUM") as psvalpool,
        ):
            # --- persistent loads -----------------------------------
            ones_sb = persist.tile([128, 128], mmdt)
            nc.sync.dma_start(out=ones_sb, in_=ones)
            bias_sb = persist.tile([1, O_LOC], mmdt)
            nc.sync.dma_start(out=bias_sb, in_=bias)

            # whole activator shard, [i%128, kt, b] layout, in KG chunks
            act_g = []
            for g in range(KG):
                t = persist.tile([128, KPG, B_LOC], mmdt, tag=f"act{g}")
                nc.sync.dma_start(
                    out=t,
                    in_=xaT[g * KPG * 128 : (g + 1) * KPG * 128, :].rearrange(
                        "(kt p) b -> p kt b", p=128
                    ),
                )
                act_g.append(t)

            def act_tile(k, j):
                return act_g[k // KPG][:, k % KPG, j * 128 : (j + 1) * 128]

            # carry row sums land here (emitted inside the o==0 body so the
            # xc DMAs queue behind the first w tiles instead of ahead of them)
            csum_sb = small.tile([128, JT], dt.float32)

            # --- main GEMM + act row sums ---------------------------
            # fp32r matmuls need an even moving/dst free count, so the
            # row-sum matmuls use N=2 (two ones columns; col 2j+1 is a
            # duplicate of 2j and ignored).
            psum_val = psvalpool.tile([128, 4 * JT], dt.float32)
            val_sb = small.tile([128, JT], dt.float32)
            for o in range(OT):
                o_sl = slice(o * 512, (o + 1) * 512)
                ps = [
                    pspool.tile([128, 512], dt.float32, tag="ps", name=f"ps_{o}_{j}")
                    for j in range(JT)
                ]
                for k in range(KT):
                    w_t = wpool.tile([128, 512], mmdt, tag="wt")
                    nc.sync.dma_start(
                        out=w_t, in_=wT[k * 128 : (k + 1) * 128, o_sl]
                    )
                    for j in range(JT):
                        nc.tensor.matmul(
                            ps[j], act_tile(k, j), w_t,
                            start=(k == 0), stop=(k == KT - 1),
                        )
                        if k == 0:
                            # bias: ones-row (K=1) x bias-row accumulate
                            nc.tensor.matmul(
                                ps[j], ones_sb[0:1, :], bias_sb[0:1, o_sl],
                                start=False, stop=False,
                            )
                        if o == 0:
                            # activator row sums: act_tile.T @ ones_cols.
                            # start=True clears has_written for the WHOLE
                            # bank, so only the first sum-MM may set it —
                            # later js overwrite-on-unset and accumulate
                            # from there.
                            nc.tensor.matmul(
                                psum_val[:, 4 * j : 4 * j + 2],
                                act_tile(k, j), ones_sb[:, 0:2],
                                start=(k == 0 and j == 0),
                                stop=(k == KT - 1 and j == JT - 1),
                                skip_group_check=True,
                            )
                if o == 0:
                    with nc.named_scope("carry_sums"):
                        for j in range(JT):
                            xc_t = xcpool.tile(
                                [128, D], dt.float32, tag="xc", name=f"xc_{j}"
                            )
                            nc.sync.dma_start(
                                out=xc_t, in_=xc[j * 128 : (j + 1) * 128, :]
                            )
                            nc.vector.reduce_sum(
                                csum_sb[:, j : j + 1], xc_t,
                                axis=mybir.AxisListType.X,
                            )
                    # val = (act_sum + carry_sum) / (2*D)
                    for j in range(JT):
                        nc.vector.tensor_scalar(
                            val_sb[:, j : j + 1], psum_val[:, 4 * j : 4 * j + 1],
                            csum_sb[:, j : j + 1], 1.0 / (2 * D),
                            op0=mybir.AluOpType.add, op1=mybir.AluOpType.mult,
                        )
                    if debug:
                        dbg_sb = small.tile([128, 6 * JT], dt.float32)
                        nc.vector.tensor_copy(dbg_sb[:, : 4 * JT], psum_val)
                        nc.vector.tensor_copy(
                            dbg_sb[:, 4 * JT : 5 * JT], csum_sb
                        )
                        nc.vector.tensor_copy(dbg_sb[:, 5 * JT :], val_sb)
                        nc.sync.dma_start(out=dbg, in_=dbg_sb)
                # --- epilogue: relu + carry broadcast + store -------
                for j in range(JT):
                    out_t = opool.tile([128, 512, 2], dt.float32, tag="out")
                    nc.vector.tensor_scalar_max(out_t[:, :, 0], ps[j], 0.0)
                    nc.vector.tensor_scalar(
                        out_t[:, :, 1], ps[j], 0.0, val_sb[:, j : j + 1],
                        op0=mybir.AluOpType.mult, op1=mybir.AluOpType.add,
                    )
                    nc.sync.dma_start(
                        out=out[j * 128 : (j + 1) * 128, o_sl, :], in_=out_t
                    )
    nc.compile()
    return nc


def _np_mmdt():
    if MM_DTYPE == "float16":
        return np.float16
    if MM_DTYPE == "bfloat16":
        import ml_dtypes

        return np.dtype(ml_dtypes.bfloat16)
    return np.float32  # float32 / float32r


def _shard_inputs(x, W, b):
    ndt = _np_mmdt()
    x = np.ascontiguousarray(x, dtype=np.float32)
    W = np.asarray(W, dtype=np.float32)
    b = np.asarray(b, dtype=np.float32)
    wT_shards = [
        np.ascontiguousarray(W[c * O_LOC : (c + 1) * O_LOC, :].T).astype(ndt)
        for c in range(F_SHARDS)
    ]
    bias_shards = [
        b[c * O_LOC : (c + 1) * O_LOC].reshape(1, O_LOC).astype(ndt)
        for c in range(F_SHARDS)
    ]
    ones = np.ones((128, 128), dtype=ndt)
    in_maps = []
    for core in range(M_SHARDS * F_SHARDS):
        r, c = core % M_SHARDS, core // M_SHARDS
        b_sl = slice(r * B_LOC, (r + 1) * B_LOC)
        in_maps.append(
            dict(
                xaT=np.ascontiguousarray(x[b_sl, :, 0].T).astype(ndt),
                xc=np.ascontiguousarray(x[b_sl, :, 1]),
                wT=wT_shards[c],
                bias=bias_shards[c],
                ones=ones,
            )
        )
    return in_maps


def _gather(results):
    out = np.empty((B, D, 2), dtype=np.float32)
    for core, r in enumerate(results):
        m, c = core % M_SHARDS, core // M_SHARDS
        out[m * B_LOC : (m + 1) * B_LOC, c * O_LOC : (c + 1) * O_LOC, :] = r["out"]
    return out


def _run(x, W, b, trace=False, **spmd_kwargs):
    in_maps = _shard_inputs(x, W, b)
    nc = _build()
    res = run_bass_kernel_spmd(
        nc, in_maps, core_ids=list(range(8)), trace=trace, **spmd_kwargs
    )
    return _gather(res.results), res


def kernel(x, W, b):
    out, _ = _run(x, W, b, trace=False)
    return out
